# revision 23
# baseline (speedup 1.0000x reference)
"""Trainium2 Bass kernel: batched FFT along axis 1 of x[64, 4096, 128] (fp32),
returning (real, imag) parts.  8-core data-parallel over the batch axis.

Algorithm (per core, 8 batches): 4-step Cooley-Tukey, N = 128*32,
n = 32*n1 + n2, k = 128*k2 + k1:

    X[128*k2 + k1] = sum_n2 W32[n2,k2] * (W4096[n2*k1] * sum_n1 W128[n1,k1]*x)

Phase 1 (per batch b, m-range mr of 32): stage-1 DFT-128 over n1 with the
  twiddle FOLDED into per-n2 stationaries PQ[n1, (comp, k1=0..63)] -- both
  components packed into one 128-wide output so every PSUM partition carries
  real data.  One f=32 matmul per n2 fills a strided column slot of a PSUM
  tile a[128, (m32, n2-32)] (n2 innermost).
Evict: one ACT copy per PSUM tile -> fp16 B-slab [128=(c,g,jm), (b2,m,n2)].
Transpose: one DVE StreamTranspose per quarter (32x32 blocks) yields
  Bd[(c,g,n2), (b2, m, jm)] fp16 on-chip -- no DRAM bounce.
Phase 2 (per quarter, jp): stage-2 DFT-32 over n2 with wide [64,128]
  conjugate-packing stationaries (direct k1 = 32g+jm and Hermitian
  k1' = 128-q rows in one pass); fp16 moving, free dim (b2, je, m).
  C tiles are dumped verbatim to DRAM; the host relabels (pure data
  movement, no arithmetic).
Nyquist rows (k1 = 64) skip the transpose: X[128k2+64] is computed directly
  from x by a single-stage matmul accumulated over n2 in PSUM.
"""

import numpy as np
from contextlib import ExitStack

import concourse.bacc as bacc
import concourse.bass as bass
import concourse.mybir as mybir
import concourse.tile as tile
from concourse.bass_utils import run_bass_kernel_spmd

N = 4096
N1, N2 = 128, 32
M = 128
B_FULL = 64
NCORES = 8
BPER = B_FULL // NCORES  # 8 batches per core

FP16 = mybir.dt.float16
FP32 = mybir.dt.float32


# ---------------------------------------------------------------- constants
def make_consts():
    n1 = np.arange(N1)
    q = np.arange(64)
    # phase-1 folded stationaries: PQ[n1, n2*128 + c*64 + q], q = k1 = 0..63
    pq = np.zeros((N1, N2 * 128), np.float32)
    for n2 in range(N2):
        theta = 2 * np.pi * np.outer(32 * n1 + n2, q) / N
        pq[:, n2 * 128: n2 * 128 + 64] = np.cos(theta)
        pq[:, n2 * 128 + 64: n2 * 128 + 128] = -np.sin(theta)

    # nyquist single-stage stationaries: NY[n1, n2*64 + c*32 + k2]
    k2 = np.arange(N2)
    nyst = np.zeros((N1, N2 * 64), np.float32)
    for n2 in range(N2):
        th = 2 * np.pi * np.outer(32 * n1 + n2, 128 * k2 + 64) / N
        nyst[:, n2 * 64: n2 * 64 + 32] = np.cos(th)
        nyst[:, n2 * 64 + 32: n2 * 64 + 64] = -np.sin(th)

    # phase-2 stationaries, direct rows only (k1 = 32g+jm, G in {0,1}); the
    # Hermitian half (k1 = 65..127) is mirrored on the host from these.
    n2v = np.arange(N2)
    a2 = 2 * np.pi * np.outer(n2v, k2) / N2
    c, s = np.cos(a2), -np.sin(a2)

    def bdiag(up):
        z = np.zeros((64, 64), np.float32)
        z[0:32, 0:32] = up
        z[32:64, 32:64] = up
        return z

    # stacked [128, 64] stationaries: rows 0..63 act on Bd_re (bd[0:64]),
    # rows 64..127 on Bd_im (bd[64:128]) -- one matmul per C component
    su = np.zeros((128, 128), np.float32)
    su[0:64, 0:64] = bdiag(c)       # C_re <- Bd_re
    su[64:128, 0:64] = bdiag(-s)    # C_re <- Bd_im
    su[0:64, 64:128] = bdiag(s)     # C_im <- Bd_re
    su[64:128, 64:128] = bdiag(c)   # C_im <- Bd_im

    return {
        "pq": pq.astype(np.float16),
        "nyst": nyst.astype(np.float16),
        "su": su.astype(np.float16),
    }


def _hand_ap(base_ap, rel_off, dims):
    return bass.AP(tensor=base_ap.tensor, offset=base_ap.offset + rel_off,
                   ap=[list(d) for d in dims])


# ---------------------------------------------------------------- program
def build_program():
    nc = bacc.Bacc("TRN2", target_bir_lowering=False, debug=False)

    x_in = nc.dram_tensor("x", [BPER, N, M], FP16, kind="ExternalInput")
    # raw C-tile dumps: outd[q, jp, po, (c, b2, je, m)]
    outd = nc.dram_tensor("outd", [4, 4, 128, 2048], FP16,
                          kind="ExternalOutput")
    nyd = nc.dram_tensor("nyd", [4, 64, 256], FP16, kind="ExternalOutput")
    pq_in = nc.dram_tensor("pq", [N1, N2 * 128], FP16, kind="ExternalInput")
    nyst_in = nc.dram_tensor("nyst", [N1, N2 * 64], FP16,
                             kind="ExternalInput")
    su_in = nc.dram_tensor("su", [128, 128], FP16, kind="ExternalInput")

    with tile.TileContext(nc) as tc, ExitStack() as ctx:
        cpool = ctx.enter_context(tc.tile_pool(name="consts", bufs=1))
        ct_pq = cpool.tile([N1, N2 * 128], FP16, tag="pq", name="ct_pq")
        ct_ny = cpool.tile([N1, N2 * 64], FP16, tag="nyst", name="ct_ny")
        ct_su = cpool.tile([128, 128], FP16, tag="su", name="ct_su")
        nc.sync.dma_start(ct_pq[:], pq_in.ap())
        nc.sync.dma_start(ct_ny[:], nyst_in.ap())
        nc.sync.dma_start(ct_su[:], su_in.ap())

        x_pool = ctx.enter_context(tc.tile_pool(name="xp", bufs=2))
        a_psum = ctx.enter_context(tc.tile_pool(name="aps", bufs=2, space="PSUM"))
        c_psum = ctx.enter_context(tc.tile_pool(name="cps", bufs=2, space="PSUM"))
        b_pool = ctx.enter_context(tc.tile_pool(name="bp", bufs=2))
        bd_pool = ctx.enter_context(tc.tile_pool(name="bdp", bufs=4))
        cs_pool = ctx.enter_context(tc.tile_pool(name="csp", bufs=3))
        ny_pool = ctx.enter_context(tc.tile_pool(name="nyp", bufs=2))

        def phase1_quarter(qq, bq, nysb):
            """batches 2qq, 2qq+1 -> B-slab [128=(c,q64), (b2, m, n2)] fp16."""
            for eb in range(2):
                b = 2 * qq + eb
                xt = x_pool.tile([N1, N2 * M], FP16, tag="x", name=f"xt_{b}")
                src = _hand_ap(x_in.ap(), b * N * M,
                               [[N2 * M, N1], [1, N2 * M]])
                nc.sync.dma_start(xt[:], src)
                for ng in range(4):   # n2 octets, contiguous psum writes
                    a = a_psum.tile([128, 1024], FP32, tag="a",
                                    name=f"a_{b}_{ng}")
                    for j in range(8):
                        n2 = 8 * ng + j
                        stat = ct_pq[:, n2 * 128:(n2 + 1) * 128]
                        mov = xt[:, n2 * M:(n2 + 1) * M]
                        nc.tensor.matmul(a[:, j * 128:(j + 1) * 128],
                                         stat, mov, start=True, stop=True)
                    # strided eviction performs the (n2, m) -> (m, n2) reorder
                    dst = bq[:].rearrange("p (b m n) -> p b m n",
                                          b=2, m=M, n=N2)[
                        :, eb, :, 8 * ng:8 * ng + 8]
                    srcp = a[:].rearrange("p (n m) -> p m n", n=8, m=M)
                    nc.scalar.copy(dst, srcp)
                # nyquist: single-stage, accumulate over n2 in PSUM
                any_ = a_psum.tile([128, 1024], FP32, tag="a", name=f"any_{b}")
                for n2 in range(N2):
                    stat = ct_ny[:, n2 * 64:(n2 + 1) * 64]
                    mov = xt[:, n2 * M:(n2 + 1) * M]
                    nc.tensor.matmul(any_[0:64, 0:128], stat, mov,
                                     start=(n2 == 0), stop=(n2 == N2 - 1))
                nc.scalar.copy(nysb[:, eb * 128:(eb + 1) * 128],
                               any_[0:64, 0:128])

        def phase2_quarter(qq, bd):
            bd4 = bd[:].rearrange("p (b m j) -> p b j m", b=2, m=M, j=N2)
            for dh in range(4):
                csb = cs_pool.tile([128, 2048], FP16, tag="csb",
                                   name=f"csb_{qq}_{dh}")
                for pp in range(2):
                    cc = c_psum.tile([128, 1024], FP32, tag="c",
                                     name=f"c_{qq}_{dh}_{pp}")
                    for jpar in range(2):
                        jp = 4 * dh + 2 * pp + jpar
                        mov = bd4[:, :, 2 * jp:2 * jp + 2, :]
                        ps = slice(64 * jpar, 64 * jpar + 64)
                        nc.tensor.matmul(cc[ps, 0:512], ct_su[:, 0:64],
                                         mov, start=True, stop=True)
                        nc.tensor.matmul(cc[ps, 512:1024], ct_su[:, 64:128],
                                         mov, start=True, stop=True)
                    dstc = csb[:, pp * 1024:(pp + 1) * 1024]
                    if pp == 0:
                        nc.scalar.copy(dstc, cc[:])
                    else:
                        nc.vector.tensor_copy(dstc, cc[:])
                dst = _hand_ap(outd.ap(), (qq * 4 + dh) * 128 * 2048,
                               [[2048, 128], [1, 2048]])
                nc.sync.dma_start(dst, csb[:])

        # ---------------- pipeline ----------------
        for qq in range(4):
            bq = b_pool.tile([128, 2 * M * N2], FP16, tag="b",
                             name=f"bq_{qq}")
            bd = bd_pool.tile([128, 2 * M * N2], FP16, tag="bd",
                              name=f"bd_{qq}")
            nysb = ny_pool.tile([64, 256], FP16, tag="nsb", name=f"nysb_{qq}")
            phase1_quarter(qq, bq, nysb)
            nc.vector.transpose(bd[:], bq[:])
            dstn = _hand_ap(nyd.ap(), qq * 64 * 256, [[256, 64], [1, 256]])
            nc.sync.dma_start(dstn, nysb[:])
            phase2_quarter(qq, bd)

    nc.compile()
    return nc


_CACHE = {}


def _get_program():
    if "nc" not in _CACHE:
        _CACHE["nc"] = build_program()
        _CACHE["consts"] = make_consts()
    return _CACHE["nc"], _CACHE["consts"]


_LAST = {}


def _k1_map():
    k1 = np.zeros((16, 2, 2), np.int64)
    jp = np.arange(16)[:, None]
    je = np.arange(2)[None, :]
    k1[:, 0, :] = 2 * jp + je
    k1[:, 1, :] = 32 + 2 * jp + je
    return k1


def _run(x: np.ndarray, trace: bool = False):
    x = np.asarray(x)
    assert x.shape == (B_FULL, N, M)
    x16 = np.ascontiguousarray(x.astype(np.float16))
    nc, consts = _get_program()
    in_maps = []
    for c in range(NCORES):
        m = {"x": np.ascontiguousarray(x16[c * BPER:(c + 1) * BPER])}
        m.update(consts)
        in_maps.append(m)
    bres = run_bass_kernel_spmd(nc, in_maps, list(range(NCORES)), trace=trace)
    _LAST["results"] = bres
    res = bres.results
    out = np.empty((2, B_FULL, N, M), np.float32)
    k1f = _k1_map().reshape(-1)
    for core in range(NCORES):
        scr = res[core]["outd"].astype(np.float32)
        nyq = res[core]["nyd"].astype(np.float32)
        # scr: [q, dh, po=(jpar G k2), f=(pp c b2 je m)]
        #   -> (c q b2 dh pp jpar G je k2 m); jp = 4dh + 2pp + jpar
        s = scr.reshape(4, 4, 2, 2, 32, 2, 2, 2, 2, M)
        p = s.transpose(6, 0, 7, 1, 5, 2, 3, 8, 4, 9)
        flat = p.reshape(2, 4, 2, 16 * 2 * 2, 32, M)
        xv = out[:, core * BPER:(core + 1) * BPER].reshape(2, 4, 2, 32, 128, M)
        xv[:, :, :, :, k1f, :] = flat.transpose(0, 1, 2, 4, 3, 5)
        # nyd: [q, (c, k2), (b2, m)]
        ny = nyq.reshape(4, 2, 32, 2, M).transpose(1, 0, 3, 2, 4)  # c q b2 k2 m
        xv[:, :, :, :, 64, :] = ny
    # Hermitian mirror: X[N-k] = conj(X[k]) (k1' = 65..127 from k1 = 63..1,
    # k2' = 31-k2); axes of full[c] are (b, k2, k1, m)
    full = out.reshape(2, B_FULL, 32, 128, M)
    full[0][:, :, 65:, :] = full[0][:, ::-1, 63:0:-1, :]
    full[1][:, :, 65:, :] = -full[1][:, ::-1, 63:0:-1, :]
    return out[0], out[1]


def kernel(x: np.ndarray):
    """x: [64, 4096, 128] fp32 -> (re, im) each [64, 4096, 128] fp32."""
    return _run(x, trace=False)


# revision 24
# speedup vs baseline: 1.0048x; 1.0048x over previous
"""Trainium2 Bass kernel: batched FFT along axis 1 of x[64, 4096, 128] (fp32),
returning (real, imag) parts.  8-core data-parallel over the batch axis.

Algorithm (per core, 8 batches): 4-step Cooley-Tukey, N = 128*32,
n = 32*n1 + n2, k = 128*k2 + k1:

    X[128*k2 + k1] = sum_n2 W32[n2,k2] * (W4096[n2*k1] * sum_n1 W128[n1,k1]*x)

Phase 1 (per batch b, m-range mr of 32): stage-1 DFT-128 over n1 with the
  twiddle FOLDED into per-n2 stationaries PQ[n1, (comp, k1=0..63)] -- both
  components packed into one 128-wide output so every PSUM partition carries
  real data.  One f=32 matmul per n2 fills a strided column slot of a PSUM
  tile a[128, (m32, n2-32)] (n2 innermost).
Evict: one ACT copy per PSUM tile -> fp16 B-slab [128=(c,g,jm), (b2,m,n2)].
Transpose: one DVE StreamTranspose per quarter (32x32 blocks) yields
  Bd[(c,g,n2), (b2, m, jm)] fp16 on-chip -- no DRAM bounce.
Phase 2 (per quarter, jp): stage-2 DFT-32 over n2 with wide [64,128]
  conjugate-packing stationaries (direct k1 = 32g+jm and Hermitian
  k1' = 128-q rows in one pass); fp16 moving, free dim (b2, je, m).
  C tiles are dumped verbatim to DRAM; the host relabels (pure data
  movement, no arithmetic).
Nyquist rows (k1 = 64) skip the transpose: X[128k2+64] is computed directly
  from x by a single-stage matmul accumulated over n2 in PSUM.
"""

import numpy as np
from contextlib import ExitStack

import concourse.bacc as bacc
import concourse.bass as bass
import concourse.mybir as mybir
import concourse.tile as tile
from concourse.bass_utils import run_bass_kernel_spmd

N = 4096
N1, N2 = 128, 32
M = 128
B_FULL = 64
NCORES = 8
BPER = B_FULL // NCORES  # 8 batches per core

FP16 = mybir.dt.float16
FP32 = mybir.dt.float32


# ---------------------------------------------------------------- constants
def make_consts():
    n1 = np.arange(N1)
    q = np.arange(64)
    # phase-1 folded stationaries: PQ[n1, n2*128 + c*64 + q], q = k1 = 0..63
    pq = np.zeros((N1, N2 * 128), np.float32)
    for n2 in range(N2):
        theta = 2 * np.pi * np.outer(32 * n1 + n2, q) / N
        pq[:, n2 * 128: n2 * 128 + 64] = np.cos(theta)
        pq[:, n2 * 128 + 64: n2 * 128 + 128] = -np.sin(theta)

    # nyquist single-stage stationaries: NY[n1, n2*64 + c*32 + k2]
    k2 = np.arange(N2)
    nyst = np.zeros((N1, N2 * 64), np.float32)
    for n2 in range(N2):
        th = 2 * np.pi * np.outer(32 * n1 + n2, 128 * k2 + 64) / N
        nyst[:, n2 * 64: n2 * 64 + 32] = np.cos(th)
        nyst[:, n2 * 64 + 32: n2 * 64 + 64] = -np.sin(th)

    # phase-2 stationaries, direct rows only (k1 = 32g+jm, G in {0,1}); the
    # Hermitian half (k1 = 65..127) is mirrored on the host from these.
    n2v = np.arange(N2)
    a2 = 2 * np.pi * np.outer(n2v, k2) / N2
    c, s = np.cos(a2), -np.sin(a2)

    def bdiag(up):
        z = np.zeros((64, 64), np.float32)
        z[0:32, 0:32] = up
        z[32:64, 32:64] = up
        return z

    # stacked [128, 64] stationaries: rows 0..63 act on Bd_re (bd[0:64]),
    # rows 64..127 on Bd_im (bd[64:128]) -- one matmul per C component
    su = np.zeros((128, 128), np.float32)
    su[0:64, 0:64] = bdiag(c)       # C_re <- Bd_re
    su[64:128, 0:64] = bdiag(-s)    # C_re <- Bd_im
    su[0:64, 64:128] = bdiag(s)     # C_im <- Bd_re
    su[64:128, 64:128] = bdiag(c)   # C_im <- Bd_im

    return {
        "pq": pq.astype(np.float16),
        "nyst": nyst.astype(np.float16),
        "su": su.astype(np.float16),
    }


def _hand_ap(base_ap, rel_off, dims):
    return bass.AP(tensor=base_ap.tensor, offset=base_ap.offset + rel_off,
                   ap=[list(d) for d in dims])


# ---------------------------------------------------------------- program
def build_program():
    nc = bacc.Bacc("TRN2", target_bir_lowering=False, debug=False)

    x_in = nc.dram_tensor("x", [BPER, N, M], FP16, kind="ExternalInput")
    # raw C-tile dumps: outd[q, jp, po, (c, b2, je, m)]
    outd = nc.dram_tensor("outd", [4, 4, 128, 2048], FP16,
                          kind="ExternalOutput")
    nyd = nc.dram_tensor("nyd", [4, 64, 256], FP16, kind="ExternalOutput")
    pq_in = nc.dram_tensor("pq", [N1, N2 * 128], FP16, kind="ExternalInput")
    nyst_in = nc.dram_tensor("nyst", [N1, N2 * 64], FP16,
                             kind="ExternalInput")
    su_in = nc.dram_tensor("su", [128, 128], FP16, kind="ExternalInput")

    with tile.TileContext(nc) as tc, ExitStack() as ctx:
        cpool = ctx.enter_context(tc.tile_pool(name="consts", bufs=1))
        ct_pq = cpool.tile([N1, N2 * 128], FP16, tag="pq", name="ct_pq")
        ct_ny = cpool.tile([N1, N2 * 64], FP16, tag="nyst", name="ct_ny")
        ct_su = cpool.tile([128, 128], FP16, tag="su", name="ct_su")
        nc.sync.dma_start(ct_pq[:], pq_in.ap())
        nc.sync.dma_start(ct_ny[:], nyst_in.ap())
        nc.sync.dma_start(ct_su[:], su_in.ap())

        x_pool = ctx.enter_context(tc.tile_pool(name="xp", bufs=2))
        a_psum = ctx.enter_context(tc.tile_pool(name="aps", bufs=2, space="PSUM"))
        c_psum = ctx.enter_context(tc.tile_pool(name="cps", bufs=2, space="PSUM"))
        b_pool = ctx.enter_context(tc.tile_pool(name="bp", bufs=2))
        bd_pool = ctx.enter_context(tc.tile_pool(name="bdp", bufs=4))
        cs_pool = ctx.enter_context(tc.tile_pool(name="csp", bufs=3))
        ny_pool = ctx.enter_context(tc.tile_pool(name="nyp", bufs=2))

        def phase1_quarter(qq, bq, nysb):
            """batches 2qq, 2qq+1 -> B-slab [128=(c,q64), (b2, m, n2)] fp16."""
            for eb in range(2):
                b = 2 * qq + eb
                xt = x_pool.tile([N1, N2 * M], FP16, tag="x", name=f"xt_{b}")
                src = _hand_ap(x_in.ap(), b * N * M,
                               [[N2 * M, N1], [1, N2 * M]])
                nc.sync.dma_start(xt[:], src)
                for ng in range(4):   # n2 octets, contiguous psum writes
                    a = a_psum.tile([128, 1024], FP32, tag="a",
                                    name=f"a_{b}_{ng}")
                    for j in range(8):
                        n2 = 8 * ng + j
                        stat = ct_pq[:, n2 * 128:(n2 + 1) * 128]
                        mov = xt[:, n2 * M:(n2 + 1) * M]
                        nc.tensor.matmul(a[:, j * 128:(j + 1) * 128],
                                         stat, mov, start=True, stop=True)
                    # strided eviction performs the (n2, m) -> (m, n2) reorder
                    dst = bq[:].rearrange("p (b m n) -> p b m n",
                                          b=2, m=M, n=N2)[
                        :, eb, :, 8 * ng:8 * ng + 8]
                    srcp = a[:].rearrange("p (n m) -> p m n", n=8, m=M)
                    nc.scalar.copy(dst, srcp)
                # nyquist: single-stage, accumulate over n2 in PSUM
                any_ = a_psum.tile([128, 1024], FP32, tag="a", name=f"any_{b}")
                for n2 in range(N2):
                    stat = ct_ny[:, n2 * 64:(n2 + 1) * 64]
                    mov = xt[:, n2 * M:(n2 + 1) * M]
                    nc.tensor.matmul(any_[0:64, 0:128], stat, mov,
                                     start=(n2 == 0), stop=(n2 == N2 - 1))
                nc.scalar.copy(nysb[:, eb * 128:(eb + 1) * 128],
                               any_[0:64, 0:128])

        def phase2_quarter(qq, bd):
            bd4 = bd[:].rearrange("p (b m j) -> p b j m", b=2, m=M, j=N2)
            for dh in range(4):
                csb = cs_pool.tile([128, 2048], FP16, tag="csb",
                                   name=f"csb_{qq}_{dh}")
                for pp in range(2):
                    cc = c_psum.tile([128, 1024], FP32, tag="c",
                                     name=f"c_{qq}_{dh}_{pp}")
                    for jpar in range(2):
                        jp = 4 * dh + 2 * pp + jpar
                        mov = bd4[:, :, 2 * jp:2 * jp + 2, :]
                        ps = slice(64 * jpar, 64 * jpar + 64)
                        nc.tensor.matmul(cc[ps, 0:512], ct_su[:, 0:64],
                                         mov, start=True, stop=True)
                        nc.tensor.matmul(cc[ps, 512:1024], ct_su[:, 64:128],
                                         mov, start=True, stop=True)
                    dstc = csb[:, pp * 1024:(pp + 1) * 1024]
                    if pp == 0:
                        nc.scalar.copy(dstc, cc[:])
                    else:
                        nc.vector.tensor_copy(dstc, cc[:])
                dst = _hand_ap(outd.ap(), (qq * 4 + dh) * 128 * 2048,
                               [[2048, 128], [1, 2048]])
                nc.sync.dma_start(dst, csb[:])

        # ---------------- pipeline ----------------
        # software-pipelined emission with a one-quarter lag: PE can run
        # phase1(q+1) while the DVE transpose of quarter q completes.
        bqs, bds, nys = {}, {}, {}
        for qq in range(4):
            bqs[qq] = b_pool.tile([128, 2 * M * N2], FP16, tag="b",
                                  name=f"bq_{qq}")
            bds[qq] = bd_pool.tile([128, 2 * M * N2], FP16, tag="bd",
                                   name=f"bd_{qq}")
            nys[qq] = ny_pool.tile([64, 256], FP16, tag="nsb",
                                   name=f"nysb_{qq}")
            phase1_quarter(qq, bqs[qq], nys[qq])
            nc.vector.transpose(bds[qq][:], bqs[qq][:])
            dstn = _hand_ap(nyd.ap(), qq * 64 * 256, [[256, 64], [1, 256]])
            nc.sync.dma_start(dstn, nys[qq][:])
            if qq >= 1:
                phase2_quarter(qq - 1, bds[qq - 1])
        phase2_quarter(3, bds[3])

    nc.compile()
    return nc


_CACHE = {}


def _get_program():
    if "nc" not in _CACHE:
        _CACHE["nc"] = build_program()
        _CACHE["consts"] = make_consts()
    return _CACHE["nc"], _CACHE["consts"]


_LAST = {}


def _k1_map():
    k1 = np.zeros((16, 2, 2), np.int64)
    jp = np.arange(16)[:, None]
    je = np.arange(2)[None, :]
    k1[:, 0, :] = 2 * jp + je
    k1[:, 1, :] = 32 + 2 * jp + je
    return k1


def _run(x: np.ndarray, trace: bool = False):
    x = np.asarray(x)
    assert x.shape == (B_FULL, N, M)
    x16 = np.ascontiguousarray(x.astype(np.float16))
    nc, consts = _get_program()
    in_maps = []
    for c in range(NCORES):
        m = {"x": np.ascontiguousarray(x16[c * BPER:(c + 1) * BPER])}
        m.update(consts)
        in_maps.append(m)
    bres = run_bass_kernel_spmd(nc, in_maps, list(range(NCORES)), trace=trace)
    _LAST["results"] = bres
    res = bres.results
    out = np.empty((2, B_FULL, N, M), np.float32)
    k1f = _k1_map().reshape(-1)
    for core in range(NCORES):
        scr = res[core]["outd"].astype(np.float32)
        nyq = res[core]["nyd"].astype(np.float32)
        # scr: [q, dh, po=(jpar G k2), f=(pp c b2 je m)]
        #   -> (c q b2 dh pp jpar G je k2 m); jp = 4dh + 2pp + jpar
        s = scr.reshape(4, 4, 2, 2, 32, 2, 2, 2, 2, M)
        p = s.transpose(6, 0, 7, 1, 5, 2, 3, 8, 4, 9)
        flat = p.reshape(2, 4, 2, 16 * 2 * 2, 32, M)
        xv = out[:, core * BPER:(core + 1) * BPER].reshape(2, 4, 2, 32, 128, M)
        xv[:, :, :, :, k1f, :] = flat.transpose(0, 1, 2, 4, 3, 5)
        # nyd: [q, (c, k2), (b2, m)]
        ny = nyq.reshape(4, 2, 32, 2, M).transpose(1, 0, 3, 2, 4)  # c q b2 k2 m
        xv[:, :, :, :, 64, :] = ny
    # Hermitian mirror: X[N-k] = conj(X[k]) (k1' = 65..127 from k1 = 63..1,
    # k2' = 31-k2); axes of full[c] are (b, k2, k1, m)
    full = out.reshape(2, B_FULL, 32, 128, M)
    full[0][:, :, 65:, :] = full[0][:, ::-1, 63:0:-1, :]
    full[1][:, :, 65:, :] = -full[1][:, ::-1, 63:0:-1, :]
    return out[0], out[1]


def kernel(x: np.ndarray):
    """x: [64, 4096, 128] fp32 -> (re, im) each [64, 4096, 128] fp32."""
    return _run(x, trace=False)


# revision 25
# speedup vs baseline: 1.0483x; 1.0433x over previous
"""Trainium2 Bass kernel: batched FFT along axis 1 of x[64, 4096, 128] (fp32),
returning (real, imag) parts.  8-core data-parallel over the batch axis.

Algorithm (per core, 8 batches): 4-step Cooley-Tukey, N = 128*32,
n = 32*n1 + n2, k = 128*k2 + k1:

    X[128*k2 + k1] = sum_n2 W32[n2,k2] * (W4096[n2*k1] * sum_n1 W128[n1,k1]*x)

Phase 1 (per batch b, m-range mr of 32): stage-1 DFT-128 over n1 with the
  twiddle FOLDED into per-n2 stationaries PQ[n1, (comp, k1=0..63)] -- both
  components packed into one 128-wide output so every PSUM partition carries
  real data.  One f=32 matmul per n2 fills a strided column slot of a PSUM
  tile a[128, (m32, n2-32)] (n2 innermost).
Evict: one ACT copy per PSUM tile -> fp16 B-slab [128=(c,g,jm), (b2,m,n2)].
Transpose: one DVE StreamTranspose per quarter (32x32 blocks) yields
  Bd[(c,g,n2), (b2, m, jm)] fp16 on-chip -- no DRAM bounce.
Phase 2 (per quarter, jp): stage-2 DFT-32 over n2 with wide [64,128]
  conjugate-packing stationaries (direct k1 = 32g+jm and Hermitian
  k1' = 128-q rows in one pass); fp16 moving, free dim (b2, je, m).
  C tiles are dumped verbatim to DRAM; the host relabels (pure data
  movement, no arithmetic).
Nyquist rows (k1 = 64) skip the transpose: X[128k2+64] is computed directly
  from x by a single-stage matmul accumulated over n2 in PSUM.
"""

import numpy as np
from contextlib import ExitStack

import concourse.bacc as bacc
import concourse.bass as bass
import concourse.mybir as mybir
import concourse.tile as tile
from concourse.bass_utils import run_bass_kernel_spmd

N = 4096
N1, N2 = 128, 32
M = 128
B_FULL = 64
NCORES = 8
BPER = B_FULL // NCORES  # 8 batches per core

FP16 = mybir.dt.float16
FP32 = mybir.dt.float32


# ---------------------------------------------------------------- constants
def make_consts():
    n1 = np.arange(N1)
    q = np.arange(64)
    # phase-1 folded stationaries: PQ[n1, n2*128 + c*64 + q], q = k1 = 0..63
    pq = np.zeros((N1, N2 * 128), np.float32)
    for n2 in range(N2):
        theta = 2 * np.pi * np.outer(32 * n1 + n2, q) / N
        pq[:, n2 * 128: n2 * 128 + 64] = np.cos(theta)
        pq[:, n2 * 128 + 64: n2 * 128 + 128] = -np.sin(theta)

    # nyquist single-stage stationaries: NY[n1, n2*64 + c*32 + k2]
    k2 = np.arange(N2)
    nyst = np.zeros((N1, N2 * 64), np.float32)
    for n2 in range(N2):
        th = 2 * np.pi * np.outer(32 * n1 + n2, 128 * k2 + 64) / N
        nyst[:, n2 * 64: n2 * 64 + 32] = np.cos(th)
        nyst[:, n2 * 64 + 32: n2 * 64 + 64] = -np.sin(th)

    # phase-2 stationaries, direct rows only (k1 = 32g+jm, G in {0,1}); the
    # Hermitian half (k1 = 65..127) is mirrored on the host from these.
    n2v = np.arange(N2)
    a2 = 2 * np.pi * np.outer(n2v, k2) / N2
    c, s = np.cos(a2), -np.sin(a2)

    def bdiag(up):
        z = np.zeros((64, 64), np.float32)
        z[0:32, 0:32] = up
        z[32:64, 32:64] = up
        return z

    # stacked [128, 64] stationaries: rows 0..63 act on Bd_re (bd[0:64]),
    # rows 64..127 on Bd_im (bd[64:128]) -- one matmul per C component
    su = np.zeros((128, 128), np.float32)
    su[0:64, 0:64] = bdiag(c)       # C_re <- Bd_re
    su[64:128, 0:64] = bdiag(-s)    # C_re <- Bd_im
    su[0:64, 64:128] = bdiag(s)     # C_im <- Bd_re
    su[64:128, 64:128] = bdiag(c)   # C_im <- Bd_im

    return {
        "pq": pq.astype(np.float16),
        "nyst": nyst.astype(np.float16),
        "su": su.astype(np.float16),
    }


def _hand_ap(base_ap, rel_off, dims):
    return bass.AP(tensor=base_ap.tensor, offset=base_ap.offset + rel_off,
                   ap=[list(d) for d in dims])


# ---------------------------------------------------------------- program
def build_program():
    nc = bacc.Bacc("TRN2", target_bir_lowering=False, debug=False)

    x_in = nc.dram_tensor("x", [BPER, N, M], FP16, kind="ExternalInput")
    # raw C-tile dumps: outd[q, jp, po, (c, b2, je, m)]
    outd = nc.dram_tensor("outd", [4, 4, 128, 2048], FP16,
                          kind="ExternalOutput")
    nyd = nc.dram_tensor("nyd", [4, 64, 256], FP16, kind="ExternalOutput")
    pq_in = nc.dram_tensor("pq", [N1, N2 * 128], FP16, kind="ExternalInput")
    nyst_in = nc.dram_tensor("nyst", [N1, N2 * 64], FP16,
                             kind="ExternalInput")
    su_in = nc.dram_tensor("su", [128, 128], FP16, kind="ExternalInput")

    with tile.TileContext(nc) as tc, ExitStack() as ctx:
        cpool = ctx.enter_context(tc.tile_pool(name="consts", bufs=1))
        ct_pq = cpool.tile([N1, N2 * 128], FP16, tag="pq", name="ct_pq")
        ct_ny = cpool.tile([N1, N2 * 64], FP16, tag="nyst", name="ct_ny")
        ct_su = cpool.tile([128, 128], FP16, tag="su", name="ct_su")
        nc.sync.dma_start(ct_pq[:], pq_in.ap())
        nc.sync.dma_start(ct_ny[:], nyst_in.ap())
        nc.sync.dma_start(ct_su[:], su_in.ap())

        x_pool = ctx.enter_context(tc.tile_pool(name="xp", bufs=2))
        a_psum = ctx.enter_context(tc.tile_pool(name="aps", bufs=2, space="PSUM"))
        c_psum = ctx.enter_context(tc.tile_pool(name="cps", bufs=2, space="PSUM"))
        b_pool = ctx.enter_context(tc.tile_pool(name="bp", bufs=2))
        bd_pool = ctx.enter_context(tc.tile_pool(name="bdp", bufs=4))
        cs_pool = ctx.enter_context(tc.tile_pool(name="csp", bufs=3))
        ny_pool = ctx.enter_context(tc.tile_pool(name="nyp", bufs=2))

        def phase1_quarter(qq, bq, nysb):
            """batches 2qq, 2qq+1 -> B-slab [128=(c,q64), (b2, m, n2)] fp16."""
            for eb in range(2):
                b = 2 * qq + eb
                xt = x_pool.tile([N1, N2 * M], FP16, tag="x", name=f"xt_{b}")
                for xh in range(2):
                    hw = N2 * M // 2
                    src = _hand_ap(x_in.ap(), b * N * M + xh * hw,
                                   [[N2 * M, N1], [1, hw]])
                    nc.sync.dma_start(xt[:, xh * hw:(xh + 1) * hw], src)
                for ng in range(4):   # n2 octets, contiguous psum writes
                    a = a_psum.tile([128, 1024], FP32, tag="a",
                                    name=f"a_{b}_{ng}")
                    for j in range(8):
                        n2 = 8 * ng + j
                        stat = ct_pq[:, n2 * 128:(n2 + 1) * 128]
                        mov = xt[:, n2 * M:(n2 + 1) * M]
                        nc.tensor.matmul(a[:, j * 128:(j + 1) * 128],
                                         stat, mov, start=True, stop=True)
                    # strided eviction performs the (n2, m) -> (m, n2) reorder
                    dst = bq[:].rearrange("p (b m n) -> p b m n",
                                          b=2, m=M, n=N2)[
                        :, eb, :, 8 * ng:8 * ng + 8]
                    srcp = a[:].rearrange("p (n m) -> p m n", n=8, m=M)
                    nc.scalar.copy(dst, srcp)
                # nyquist: single-stage, accumulate over n2 in PSUM
                any_ = a_psum.tile([128, 1024], FP32, tag="a", name=f"any_{b}")
                for n2 in range(N2):
                    stat = ct_ny[:, n2 * 64:(n2 + 1) * 64]
                    mov = xt[:, n2 * M:(n2 + 1) * M]
                    nc.tensor.matmul(any_[0:64, 0:128], stat, mov,
                                     start=(n2 == 0), stop=(n2 == N2 - 1))
                nc.scalar.copy(nysb[:, eb * 128:(eb + 1) * 128],
                               any_[0:64, 0:128])

        def phase2_quarter(qq, bd):
            bd4 = bd[:].rearrange("p (b m j) -> p b j m", b=2, m=M, j=N2)
            for dh in range(4):
                csb = cs_pool.tile([128, 2048], FP16, tag="csb",
                                   name=f"csb_{qq}_{dh}")
                for pp in range(2):
                    cc = c_psum.tile([128, 1024], FP32, tag="c",
                                     name=f"c_{qq}_{dh}_{pp}")
                    for jpar in range(2):
                        jp = 4 * dh + 2 * pp + jpar
                        mov = bd4[:, :, 2 * jp:2 * jp + 2, :]
                        ps = slice(64 * jpar, 64 * jpar + 64)
                        nc.tensor.matmul(cc[ps, 0:512], ct_su[:, 0:64],
                                         mov, start=True, stop=True)
                        nc.tensor.matmul(cc[ps, 512:1024], ct_su[:, 64:128],
                                         mov, start=True, stop=True)
                    dstc = csb[:, pp * 1024:(pp + 1) * 1024]
                    if pp == 0:
                        nc.scalar.copy(dstc, cc[:])
                    else:
                        nc.vector.tensor_copy(dstc, cc[:])
                dst = _hand_ap(outd.ap(), (qq * 4 + dh) * 128 * 2048,
                               [[2048, 128], [1, 2048]])
                nc.sync.dma_start(dst, csb[:])

        # ---------------- pipeline ----------------
        # software-pipelined emission with a one-quarter lag: PE can run
        # phase1(q+1) while the DVE transpose of quarter q completes.
        bqs, bds, nys = {}, {}, {}
        for qq in range(4):
            bqs[qq] = b_pool.tile([128, 2 * M * N2], FP16, tag="b",
                                  name=f"bq_{qq}")
            bds[qq] = bd_pool.tile([128, 2 * M * N2], FP16, tag="bd",
                                   name=f"bd_{qq}")
            nys[qq] = ny_pool.tile([64, 256], FP16, tag="nsb",
                                   name=f"nysb_{qq}")
            phase1_quarter(qq, bqs[qq], nys[qq])
            nc.vector.transpose(bds[qq][:], bqs[qq][:])
            dstn = _hand_ap(nyd.ap(), qq * 64 * 256, [[256, 64], [1, 256]])
            nc.sync.dma_start(dstn, nys[qq][:])
            if qq >= 1:
                phase2_quarter(qq - 1, bds[qq - 1])
        phase2_quarter(3, bds[3])

    nc.compile()
    return nc


_CACHE = {}


def _get_program():
    if "nc" not in _CACHE:
        _CACHE["nc"] = build_program()
        _CACHE["consts"] = make_consts()
    return _CACHE["nc"], _CACHE["consts"]


_LAST = {}


def _k1_map():
    k1 = np.zeros((16, 2, 2), np.int64)
    jp = np.arange(16)[:, None]
    je = np.arange(2)[None, :]
    k1[:, 0, :] = 2 * jp + je
    k1[:, 1, :] = 32 + 2 * jp + je
    return k1


def _run(x: np.ndarray, trace: bool = False):
    x = np.asarray(x)
    assert x.shape == (B_FULL, N, M)
    x16 = np.ascontiguousarray(x.astype(np.float16))
    nc, consts = _get_program()
    in_maps = []
    for c in range(NCORES):
        m = {"x": np.ascontiguousarray(x16[c * BPER:(c + 1) * BPER])}
        m.update(consts)
        in_maps.append(m)
    bres = run_bass_kernel_spmd(nc, in_maps, list(range(NCORES)), trace=trace)
    _LAST["results"] = bres
    res = bres.results
    out = np.empty((2, B_FULL, N, M), np.float32)
    k1f = _k1_map().reshape(-1)
    for core in range(NCORES):
        scr = res[core]["outd"].astype(np.float32)
        nyq = res[core]["nyd"].astype(np.float32)
        # scr: [q, dh, po=(jpar G k2), f=(pp c b2 je m)]
        #   -> (c q b2 dh pp jpar G je k2 m); jp = 4dh + 2pp + jpar
        s = scr.reshape(4, 4, 2, 2, 32, 2, 2, 2, 2, M)
        p = s.transpose(6, 0, 7, 1, 5, 2, 3, 8, 4, 9)
        flat = p.reshape(2, 4, 2, 16 * 2 * 2, 32, M)
        xv = out[:, core * BPER:(core + 1) * BPER].reshape(2, 4, 2, 32, 128, M)
        xv[:, :, :, :, k1f, :] = flat.transpose(0, 1, 2, 4, 3, 5)
        # nyd: [q, (c, k2), (b2, m)]
        ny = nyq.reshape(4, 2, 32, 2, M).transpose(1, 0, 3, 2, 4)  # c q b2 k2 m
        xv[:, :, :, :, 64, :] = ny
    # Hermitian mirror: X[N-k] = conj(X[k]) (k1' = 65..127 from k1 = 63..1,
    # k2' = 31-k2); axes of full[c] are (b, k2, k1, m)
    full = out.reshape(2, B_FULL, 32, 128, M)
    full[0][:, :, 65:, :] = full[0][:, ::-1, 63:0:-1, :]
    full[1][:, :, 65:, :] = -full[1][:, ::-1, 63:0:-1, :]
    return out[0], out[1]


def kernel(x: np.ndarray):
    """x: [64, 4096, 128] fp32 -> (re, im) each [64, 4096, 128] fp32."""
    return _run(x, trace=False)
